# revision 26
# baseline (speedup 1.0000x reference)
"""MinGRU Trainium2 kernel (nn_MinGRU_60421599920446), v5.

Math (per batch row):
    vz[s,h] = x[s,:] @ w_z^T + bz      vh[s,h] = x[s,:] @ w_h^T + bh
    z = sigmoid(vz); h_t = (1-z_t)*h_{t-1} + z_t*vh_t   (scan over s)

Strategy: data-parallel over batch, 1 row per NeuronCore (8 cores).
All tensors live in the transposed domain [channel on partitions, S on free]
end to end: the host pre-transposes x to xT (bf16) and post-transposes the
returned hT, so the kernel does NO on-chip transposes and moves only bf16
over HBM (8 MB/core).

Per-engine assignment (hardware-measured rates):
    PE : 128 bf16 matmuls N=512 (vz, vh) + a few warm-up dummies so the HAM
         clock gate is at 8/8 before the first real matmul
    ACT: z = Sigmoid(vz+bz), hbar = Identity(vh+bh)  [PSUM->SBUF, bf16 out];
         also triggers the small weight DMAs (Scalar is a HWDGE engine)
    DVE: b = z*hbar (tt bf16 2x), a = 1-z (ts bf16 4x),
         tensor_tensor_scan (2 cyc/col — the overall floor), per scan-group
    GPSIMD: idle (its SBUF port is shared with the DVE; any gpsimd compute op
         measured 2-3x slower under DVE load and slowed the DVE as well)
    DMA: xT in (4 MB) + hT out (4 MB), HWDGE. All x-in triggers are enqueued
         on Sync BEFORE any output trigger: output triggers wait on scans, so
         interleaving them would head-of-line-block later x loads.

The scan is chained across groups ([1,1,2,2,2] chunks per m-half: small
first groups so the DVE starts ~10 us earlier) via the previous group's
last column; m-halves (two 128-channel groups) run in two outer passes so
PSUM holds vz/vh [128,1024] double-buffered (8 banks exactly).
"""

import numpy as np
from contextlib import ExitStack

B, S, D, H = 8, 8192, 256, 256
N_CORES = 8
CHUNK = 1024          # PSUM tile width (2 banks) and ACT instruction width
WARM_MM = 8           # PE warm-up dummy matmuls (N=512)

_CACHE = {}


def _groups(nchunk, m):
    """Scan-group sizes (in chunks): m=0 gets small lead-in groups so the
    DVE starts early; m=1's upstream is already ahead, so bigger groups cut
    per-instruction overhead. Both end small so the output tail is short."""
    if nchunk == 8:
        return [1, 1, 2, 2, 1, 1] if m == 0 else [2, 2, 2, 1, 1]
    return [1] * nchunk


def _build(seq_len, chunk):
    """Build + compile the single-core SPMD Bass program."""
    import concourse.bacc as bacc
    import concourse.tile as tile
    import concourse.mybir as mybir

    dt = mybir.dt
    f32 = dt.float32
    bf16 = dt.bfloat16
    AF = mybir.ActivationFunctionType
    OP = mybir.AluOpType

    assert chunk % 512 == 0 and seq_len % chunk == 0
    nchunk = seq_len // chunk

    nc = bacc.Bacc("TRN2", target_bir_lowering=False, debug=False)

    xT_d = nc.dram_tensor("xT", [2, 128, seq_len], bf16, kind="ExternalInput").ap()
    wz_d = nc.dram_tensor("wz", [2, 128, H], bf16, kind="ExternalInput").ap()
    wh_d = nc.dram_tensor("wh", [2, 128, H], bf16, kind="ExternalInput").ap()
    # packed per-partition columns: [half m][128][h0, bz, bh, -bz]
    cols_d = nc.dram_tensor("cols", [2, 128, 4], f32, kind="ExternalInput").ap()
    out_d = nc.dram_tensor("out", [2, 128, seq_len], bf16, kind="ExternalOutput").ap()

    with tile.TileContext(nc) as tc, ExitStack() as ctx:
        const = ctx.enter_context(tc.tile_pool(name="const", bufs=1))
        xin = ctx.enter_context(tc.tile_pool(name="xin", bufs=1))
        zp = ctx.enter_context(tc.tile_pool(name="z", bufs=4))
        hbp = ctx.enter_context(tc.tile_pool(name="hb", bufs=4))
        ap_ = ctx.enter_context(tc.tile_pool(name="a", bufs=3))
        bp = ctx.enter_context(tc.tile_pool(name="b", bufs=3))
        hp = ctx.enter_context(tc.tile_pool(name="h", bufs=6))
        vzp = ctx.enter_context(tc.tile_pool(name="vz", bufs=2, space="PSUM"))
        vhp = ctx.enter_context(tc.tile_pool(name="vh", bufs=2, space="PSUM"))

        # ---- PE warm-up: dummy matmuls into the first vz tile's corner so
        # the HAM clock gate reaches 8/8 while the x DMAs are in flight.
        scratch = const.tile([128, 512], bf16, tag="scratch")
        nc.vector.memset(scratch[:], 0.0)
        vz_first = vzp.tile([128, chunk], f32, tag="vz", name="vz_first")
        for _ in range(WARM_MM):
            nc.tensor.matmul(vz_first[:, 0:512], scratch[:, 0:128],
                             scratch[:], start=True, stop=True)

        # ---- x-in DMA triggers: the first two chunks' half-strips are
        # split across BOTH HWDGE queues (Sync + Scalar) so their triggers
        # issue in parallel instead of serializing at ~0.6 us each; the
        # remaining chunks ride Sync ahead of any output trigger.
        xc = [[xin.tile([128, chunk], bf16, tag=f"x{k}_{c}", name=f"x{k}_{c}")
               for c in range(nchunk)] for k in range(2)]
        hw_ = chunk // 2

        def x_strip(q, k, c, s_):
            q.dma_start(xc[k][c][:, s_ * hw_:(s_ + 1) * hw_],
                        xT_d[k, :, c * chunk + s_ * hw_:
                             c * chunk + (s_ + 1) * hw_])

        wz, wh, cols = [], [], []
        for k in range(2):
            x_strip(nc.sync, k, 0, 0)
            x_strip(nc.scalar, k, 0, 1)
        # weights next on Scalar (needed by the first LDWEIGHTS ~11 us)
        for k in range(2):
            tz = const.tile([128, H], bf16, tag=f"wz{k}")
            nc.scalar.dma_start(tz[:], wz_d[k])
            wz.append(tz)
            th = const.tile([128, H], bf16, tag=f"wh{k}")
            nc.scalar.dma_start(th[:], wh_d[k])
            wh.append(th)
        for k in range(2):
            x_strip(nc.sync, k, 1, 0)
            x_strip(nc.scalar, k, 1, 1)
        for m in range(2):
            t = const.tile([128, 4], f32, tag=f"cols{m}")
            nc.scalar.dma_start(t[:], cols_d[m])
            cols.append(t)
        for c in range(2, nchunk):
            for k in range(2):
                nc.sync.dma_start(xc[k][c][:],
                                  xT_d[k, :, c * chunk:(c + 1) * chunk])

        for m in range(2):
            groups = _groups(nchunk, m)
            h_prev = None
            c = 0
            for gi, g in enumerate(groups):
                gsz = g * chunk
                z_g = zp.tile([128, gsz], bf16, tag="z", name=f"z{m}_{gi}")
                hb_g = hbp.tile([128, gsz], bf16, tag="hb", name=f"hb{m}_{gi}")
                for j in range(g):
                    vz = (vz_first if (m == 0 and c == 0) else
                          vzp.tile([128, chunk], f32, tag="vz",
                                   name=f"vz{m}_{c}"))
                    vh = vhp.tile([128, chunk], f32, tag="vh", name=f"vh{m}_{c}")
                    for k in range(2):
                        for s2 in range(chunk // 512):
                            nc.tensor.matmul(
                                vz[:, s2 * 512:(s2 + 1) * 512],
                                wz[k][:, m * 128:(m + 1) * 128],
                                xc[k][c][:, s2 * 512:(s2 + 1) * 512],
                                start=(k == 0), stop=(k == 1))
                    for k in range(2):
                        for s2 in range(chunk // 512):
                            nc.tensor.matmul(
                                vh[:, s2 * 512:(s2 + 1) * 512],
                                wh[k][:, m * 128:(m + 1) * 128],
                                xc[k][c][:, s2 * 512:(s2 + 1) * 512],
                                start=(k == 0), stop=(k == 1))
                    off = j * chunk
                    nc.scalar.activation(z_g[:, off:off + chunk], vz[:],
                                         AF.Sigmoid, bias=cols[m][:, 1:2],
                                         scale=1.0)
                    nc.scalar.activation(hb_g[:, off:off + chunk], vh[:],
                                         AF.Identity, bias=cols[m][:, 2:3],
                                         scale=1.0)
                    if not (m == 0 and c <= 1):
                        for _ in range(3):
                            nc.tensor.matmul(vz[:, 0:512], scratch[:, 0:128],
                                             scratch[:], start=True, stop=True)
                    c += 1

                a_g = ap_.tile([128, gsz], bf16, tag="a", name=f"a{m}_{gi}")
                nc.vector.tensor_scalar(a_g[:], z_g[:], -1.0, 1.0,
                                        op0=OP.mult, op1=OP.add)
                b_g = bp.tile([128, gsz], bf16, tag="b", name=f"b{m}_{gi}")
                nc.vector.tensor_tensor(b_g[:], z_g[:], hb_g[:], op=OP.mult)
                h = hp.tile([128, gsz], bf16, tag="h", name=f"h{m}_{gi}")
                init = (cols[m][:, 0:1] if gi == 0
                        else h_prev[:, h_prev.shape[1] - 1:h_prev.shape[1]])
                base = (c - g) * chunk
                if gi == len(groups) - 1:
                    # final group: two chained half-scans so the last output
                    # DMA overlaps the second scan. Only m=1 (kernel end) may
                    # use the Scalar queue — earlier scan-gated triggers there
                    # would head-of-line-block later ACTIVATEs.
                    dma_q = nc.scalar if m == 1 else nc.sync
                    hw_ = gsz // 2
                    nc.vector.tensor_tensor_scan(
                        h[:, :hw_], a_g[:, :hw_], b_g[:, :hw_], init,
                        op0=OP.mult, op1=OP.add)
                    dma_q.dma_start(out_d[m, :, base:base + hw_],
                                    h[:, :hw_])
                    nc.vector.tensor_tensor_scan(
                        h[:, hw_:], a_g[:, hw_:], b_g[:, hw_:],
                        h[:, hw_ - 1:hw_], op0=OP.mult, op1=OP.add)
                    dma_q.dma_start(out_d[m, :, base + hw_:base + gsz],
                                    h[:, hw_:])
                else:
                    nc.vector.tensor_tensor_scan(h[:], a_g[:], b_g[:], init,
                                                 op0=OP.mult, op1=OP.add)
                    w = 1024 if gsz % 1024 == 0 else gsz
                    for s in range(gsz // w):
                        nc.sync.dma_start(
                            out_d[m, :, base + s * w: base + (s + 1) * w],
                            h[:, s * w:(s + 1) * w])
                h_prev = h

    nc.compile()
    return nc


def _get(seq_len, chunk):
    key = (seq_len, chunk)
    if key not in _CACHE:
        _CACHE[key] = _build(seq_len, chunk)
    return _CACHE[key]


def _make_in_maps(x, h0, w_h_w, w_h_b, w_z_w, w_z_b, n_cores=N_CORES):
    import ml_dtypes
    bf16 = ml_dtypes.bfloat16
    wzT = np.asarray(w_z_w, np.float32).T.astype(bf16).reshape(2, 128, H)
    whT = np.asarray(w_h_w, np.float32).T.astype(bf16).reshape(2, 128, H)
    bz = np.asarray(w_z_b, np.float32).reshape(2, 128)
    bh = np.asarray(w_h_b, np.float32).reshape(2, 128)
    in_maps = []
    for i in range(n_cores):
        h0c = np.asarray(h0[i, 0], np.float32).reshape(2, 128)
        cols = np.stack([h0c, bz, bh, -bz], axis=-1)  # [2,128,4]
        xT = np.ascontiguousarray(np.asarray(x[i], np.float32).T).astype(bf16)
        in_maps.append({
            "xT": np.ascontiguousarray(xT.reshape(2, 128, -1)),
            "wz": np.ascontiguousarray(wzT),
            "wh": np.ascontiguousarray(whT),
            "cols": np.ascontiguousarray(cols),
        })
    return in_maps


def kernel(x, h0, w_h_w, w_h_b, w_z_w, w_z_b):
    from concourse.bass_utils import run_bass_kernel_spmd

    nc = _get(S, CHUNK)
    in_maps = _make_in_maps(x, h0, w_h_w, w_h_b, w_z_w, w_z_b)
    res = run_bass_kernel_spmd(nc, in_maps, list(range(N_CORES)))
    out = np.empty((N_CORES, S, H), dtype=np.float32)
    for i in range(N_CORES):
        hT = np.asarray(res.results[i]["out"]).reshape(H, S)
        out[i] = hT.astype(np.float32).T
    return out


# revision 27
# speedup vs baseline: 1.0799x; 1.0799x over previous
"""MinGRU Trainium2 kernel (nn_MinGRU_60421599920446), v5.

Math (per batch row):
    vz[s,h] = x[s,:] @ w_z^T + bz      vh[s,h] = x[s,:] @ w_h^T + bh
    z = sigmoid(vz); h_t = (1-z_t)*h_{t-1} + z_t*vh_t   (scan over s)

Strategy: data-parallel over batch, 1 row per NeuronCore (8 cores).
All tensors live in the transposed domain [channel on partitions, S on free]
end to end: the host pre-transposes x to xT (bf16) and post-transposes the
returned hT, so the kernel does NO on-chip transposes and moves only bf16
over HBM (8 MB/core).

Per-engine assignment (hardware-measured rates):
    PE : 128 bf16 matmuls N=512 (vz, vh) + a few warm-up dummies so the HAM
         clock gate is at 8/8 before the first real matmul
    ACT: z = Sigmoid(vz+bz), hbar = Identity(vh+bh)  [PSUM->SBUF, bf16 out];
         also triggers the small weight DMAs (Scalar is a HWDGE engine)
    DVE: b = z*hbar (tt bf16 2x), a = 1-z (ts bf16 4x),
         tensor_tensor_scan (2 cyc/col — the overall floor), per scan-group
    GPSIMD: idle (its SBUF port is shared with the DVE; any gpsimd compute op
         measured 2-3x slower under DVE load and slowed the DVE as well)
    DMA: xT in (4 MB) + hT out (4 MB), HWDGE. All x-in triggers are enqueued
         on Sync BEFORE any output trigger: output triggers wait on scans, so
         interleaving them would head-of-line-block later x loads.

The scan is chained across groups ([1,1,2,2,2] chunks per m-half: small
first groups so the DVE starts ~10 us earlier) via the previous group's
last column; m-halves (two 128-channel groups) run in two outer passes so
PSUM holds vz/vh [128,1024] double-buffered (8 banks exactly).
"""

import numpy as np
from contextlib import ExitStack

B, S, D, H = 8, 8192, 256, 256
N_CORES = 8
CHUNK = 1024          # PSUM tile width (2 banks) and ACT instruction width
WARM_MM = 8           # PE warm-up dummy matmuls (N=512)

_CACHE = {}


def _groups(nchunk, m):
    """Scan-group sizes (in chunks): m=0 gets small lead-in groups so the
    DVE starts early; m=1's upstream is already ahead, so bigger groups cut
    per-instruction overhead. Both end small so the output tail is short."""
    if nchunk == 8:
        return [1, 1, 2, 2, 1, 1] if m == 0 else [2, 2, 2, 1, 1]
    return [1] * nchunk


def _build(seq_len, chunk):
    """Build + compile the single-core SPMD Bass program."""
    import concourse.bacc as bacc
    import concourse.tile as tile
    import concourse.mybir as mybir

    dt = mybir.dt
    f32 = dt.float32
    bf16 = dt.bfloat16
    AF = mybir.ActivationFunctionType
    OP = mybir.AluOpType

    assert chunk % 512 == 0 and seq_len % chunk == 0
    nchunk = seq_len // chunk

    nc = bacc.Bacc("TRN2", target_bir_lowering=False, debug=False)

    xT_d = nc.dram_tensor("xT", [2, 128, seq_len], bf16, kind="ExternalInput").ap()
    wz_d = nc.dram_tensor("wz", [2, 128, H], bf16, kind="ExternalInput").ap()
    wh_d = nc.dram_tensor("wh", [2, 128, H], bf16, kind="ExternalInput").ap()
    # packed per-partition columns: [half m][128][h0, bz, bh, -bz]
    cols_d = nc.dram_tensor("cols", [2, 128, 4], f32, kind="ExternalInput").ap()
    out_d = nc.dram_tensor("out", [2, 128, seq_len], bf16, kind="ExternalOutput").ap()

    with tile.TileContext(nc) as tc, ExitStack() as ctx:
        const = ctx.enter_context(tc.tile_pool(name="const", bufs=1))
        xin = ctx.enter_context(tc.tile_pool(name="xin", bufs=1))
        zp = ctx.enter_context(tc.tile_pool(name="z", bufs=4))
        hbp = ctx.enter_context(tc.tile_pool(name="hb", bufs=4))
        ap_ = ctx.enter_context(tc.tile_pool(name="a", bufs=3))
        bp = ctx.enter_context(tc.tile_pool(name="b", bufs=3))
        hp = ctx.enter_context(tc.tile_pool(name="h", bufs=6))
        vzp = ctx.enter_context(tc.tile_pool(name="vz", bufs=2, space="PSUM"))
        vhp = ctx.enter_context(tc.tile_pool(name="vh", bufs=2, space="PSUM"))

        # ---- PE warm-up: dummy matmuls into the first vz tile's corner so
        # the HAM clock gate reaches 8/8 while the x DMAs are in flight.
        scratch = const.tile([128, 512], bf16, tag="scratch")
        nc.vector.memset(scratch[:], 0.0)
        vz_first = vzp.tile([128, chunk], f32, tag="vz", name="vz_first")
        for _ in range(WARM_MM):
            nc.tensor.matmul(vz_first[:, 0:512], scratch[:, 0:128],
                             scratch[:], start=True, stop=True)

        # ---- all x-in DMA triggers up front on the Sync queue (no deps)
        xc = [[xin.tile([128, chunk], bf16, tag=f"x{k}_{c}", name=f"x{k}_{c}")
               for c in range(nchunk)] for k in range(2)]
        for c in range(nchunk):
            for k in range(2):
                if c < 2:
                    hw_ = chunk // 2
                    for s_ in range(2):
                        nc.sync.dma_start(
                            xc[k][c][:, s_ * hw_:(s_ + 1) * hw_],
                            xT_d[k, :, c * chunk + s_ * hw_:
                                 c * chunk + (s_ + 1) * hw_])
                else:
                    nc.sync.dma_start(xc[k][c][:],
                                      xT_d[k, :, c * chunk:(c + 1) * chunk])

        # ---- small constant DMAs via the Scalar queue (also HWDGE-capable)
        cols = []
        for m in range(2):
            t = const.tile([128, 4], f32, tag=f"cols{m}")
            nc.scalar.dma_start(t[:], cols_d[m])
            cols.append(t)
        wz, wh = [], []
        for k in range(2):
            tz = const.tile([128, H], bf16, tag=f"wz{k}")
            nc.scalar.dma_start(tz[:], wz_d[k])
            wz.append(tz)
            th = const.tile([128, H], bf16, tag=f"wh{k}")
            nc.scalar.dma_start(th[:], wh_d[k])
            wh.append(th)

        for m in range(2):
            groups = _groups(nchunk, m)
            h_prev = None
            c = 0
            for gi, g in enumerate(groups):
                gsz = g * chunk
                z_g = zp.tile([128, gsz], bf16, tag="z", name=f"z{m}_{gi}")
                hb_g = hbp.tile([128, gsz], bf16, tag="hb", name=f"hb{m}_{gi}")
                for j in range(g):
                    vz = (vz_first if (m == 0 and c == 0) else
                          vzp.tile([128, chunk], f32, tag="vz",
                                   name=f"vz{m}_{c}"))
                    vh = vhp.tile([128, chunk], f32, tag="vh", name=f"vh{m}_{c}")
                    for k in range(2):
                        for s2 in range(chunk // 512):
                            nc.tensor.matmul(
                                vz[:, s2 * 512:(s2 + 1) * 512],
                                wz[k][:, m * 128:(m + 1) * 128],
                                xc[k][c][:, s2 * 512:(s2 + 1) * 512],
                                start=(k == 0), stop=(k == 1))
                    for k in range(2):
                        for s2 in range(chunk // 512):
                            nc.tensor.matmul(
                                vh[:, s2 * 512:(s2 + 1) * 512],
                                wh[k][:, m * 128:(m + 1) * 128],
                                xc[k][c][:, s2 * 512:(s2 + 1) * 512],
                                start=(k == 0), stop=(k == 1))
                    off = j * chunk
                    nc.scalar.activation(z_g[:, off:off + chunk], vz[:],
                                         AF.Sigmoid, bias=cols[m][:, 1:2],
                                         scale=1.0)
                    nc.scalar.activation(hb_g[:, off:off + chunk], vh[:],
                                         AF.Identity, bias=cols[m][:, 2:3],
                                         scale=1.0)
                    if not (m == 0 and c <= 1):
                        for _ in range(3):
                            nc.tensor.matmul(vz[:, 0:512], scratch[:, 0:128],
                                             scratch[:], start=True, stop=True)
                    c += 1

                a_g = ap_.tile([128, gsz], bf16, tag="a", name=f"a{m}_{gi}")
                nc.vector.tensor_scalar(a_g[:], z_g[:], -1.0, 1.0,
                                        op0=OP.mult, op1=OP.add)
                b_g = bp.tile([128, gsz], bf16, tag="b", name=f"b{m}_{gi}")
                nc.vector.tensor_tensor(b_g[:], z_g[:], hb_g[:], op=OP.mult)
                h = hp.tile([128, gsz], bf16, tag="h", name=f"h{m}_{gi}")
                init = (cols[m][:, 0:1] if gi == 0
                        else h_prev[:, h_prev.shape[1] - 1:h_prev.shape[1]])
                base = (c - g) * chunk
                if gi == len(groups) - 1:
                    # final group: two chained half-scans so the last output
                    # DMA overlaps the second scan. Only m=1 (kernel end) may
                    # use the Scalar queue — earlier scan-gated triggers there
                    # would head-of-line-block later ACTIVATEs.
                    dma_q = nc.scalar if m == 1 else nc.sync
                    hw_ = gsz // 2
                    nc.vector.tensor_tensor_scan(
                        h[:, :hw_], a_g[:, :hw_], b_g[:, :hw_], init,
                        op0=OP.mult, op1=OP.add)
                    dma_q.dma_start(out_d[m, :, base:base + hw_],
                                    h[:, :hw_])
                    nc.vector.tensor_tensor_scan(
                        h[:, hw_:], a_g[:, hw_:], b_g[:, hw_:],
                        h[:, hw_ - 1:hw_], op0=OP.mult, op1=OP.add)
                    dma_q.dma_start(out_d[m, :, base + hw_:base + gsz],
                                    h[:, hw_:])
                else:
                    nc.vector.tensor_tensor_scan(h[:], a_g[:], b_g[:], init,
                                                 op0=OP.mult, op1=OP.add)
                    w = 1024 if gsz % 1024 == 0 else gsz
                    for s in range(gsz // w):
                        nc.sync.dma_start(
                            out_d[m, :, base + s * w: base + (s + 1) * w],
                            h[:, s * w:(s + 1) * w])
                h_prev = h

    nc.compile()
    return nc


def _get(seq_len, chunk):
    key = (seq_len, chunk)
    if key not in _CACHE:
        _CACHE[key] = _build(seq_len, chunk)
    return _CACHE[key]


def _make_in_maps(x, h0, w_h_w, w_h_b, w_z_w, w_z_b, n_cores=N_CORES):
    import ml_dtypes
    bf16 = ml_dtypes.bfloat16
    wzT = np.asarray(w_z_w, np.float32).T.astype(bf16).reshape(2, 128, H)
    whT = np.asarray(w_h_w, np.float32).T.astype(bf16).reshape(2, 128, H)
    bz = np.asarray(w_z_b, np.float32).reshape(2, 128)
    bh = np.asarray(w_h_b, np.float32).reshape(2, 128)
    in_maps = []
    for i in range(n_cores):
        h0c = np.asarray(h0[i, 0], np.float32).reshape(2, 128)
        cols = np.stack([h0c, bz, bh, -bz], axis=-1)  # [2,128,4]
        xT = np.ascontiguousarray(np.asarray(x[i], np.float32).T).astype(bf16)
        in_maps.append({
            "xT": np.ascontiguousarray(xT.reshape(2, 128, -1)),
            "wz": np.ascontiguousarray(wzT),
            "wh": np.ascontiguousarray(whT),
            "cols": np.ascontiguousarray(cols),
        })
    return in_maps


def kernel(x, h0, w_h_w, w_h_b, w_z_w, w_z_b):
    from concourse.bass_utils import run_bass_kernel_spmd

    nc = _get(S, CHUNK)
    in_maps = _make_in_maps(x, h0, w_h_w, w_h_b, w_z_w, w_z_b)
    res = run_bass_kernel_spmd(nc, in_maps, list(range(N_CORES)))
    out = np.empty((N_CORES, S, H), dtype=np.float32)
    for i in range(N_CORES):
        hT = np.asarray(res.results[i]["out"]).reshape(H, S)
        out[i] = hT.astype(np.float32).T
    return out
